# revision 1
# baseline (speedup 1.0000x reference)
"""Trainium2 Bass kernel for nn_Boftrainer_48284022342309 (vq_codebook).

Data-parallel over batch: 64 images -> 8 cores x 8 images each.

Layout conventions (chosen so every SB->SB rhs-build DMA moves
partition-contiguous blocks):
  activation partitions: (px outer, channel inner):  p = px*C + c
  conv rhs partitions:   (j outer, channel inner):   p = j*C + c
  second image of a pair lives at partition offset 64.

Per-core pipeline (everything stays on-chip after the initial loads):
  conv1  host-im2col rhs [54=(c,dy,j), (b,y,xb)] fp16; one matmul per
         (img, y-half), M=64=(px4,o16) multipixel columns.
  conv2  rhs [96=(j6,c16), (y66,xb16)] via 6 contiguous SB->SB DMAs;
         3 dy-accumulated matmuls per image, image pair packed in PSUM.
  pool   pool-y: tensor_tensor max on free-strided PSUM views;
         pool-x: stream_shuffle(group half-swap) + fused relu/max.
  conv3  rhs [64=(j4,c16), (Y34,xb16)]; 3 dy matmuls M=48=(px2,o24).
  conv4  rhs [96=(j4,c24), (y34,xb16)]; px-split matmuls M=16 put each
         (img,px) f-slot at a 32-aligned partition base.
  RBF    f' = [f(16); f2; 1] rows per slot; one fp16 matmul against the
         sigma-augmented codebook yields -sigma*dist2 in PSUM; ACT Exp
         with fused accum_out gives a and S = sum_k a; reciprocal;
         pooled[1,512] += R^T @ a with PSUM accumulation per image.
  MLP    pooled moved into [128,(q,b)] via AP-reordering DMAs; two small
         matmuls; biases via per-partition ACT bias.
"""
import sys
sys.path.insert(0, "/opt/trn_rl_repo")

import contextlib

import numpy as np

import concourse.bass as bass
import concourse.tile as tile
from concourse import bacc, mybir
from concourse import bass_utils

F32 = mybir.dt.float32
F16 = mybir.dt.float16

N_CORES = 8
B_CORE = 8
H = W = 64
HP = 32
KCB = 512

AF = mybir.ActivationFunctionType
ALU = mybir.AluOpType


# --------------------------------------------------------------------------
# host-side transforms (numpy)
# --------------------------------------------------------------------------

def _conv_weight_mp(w, P, J):
    """w [O,C,3,3] -> per-dy lhsT [J*C, P*O]:
    W[dy][j*C+c, px*O+o] = w[o,c,dy,j-px] if 0<=j-px<=2 else 0."""
    O, C = w.shape[0], w.shape[1]
    out = np.zeros((3, J * C, P * O), np.float32)
    for dy in range(3):
        for j in range(J):
            for px in range(P):
                dx = j - px
                if 0 <= dx <= 2:
                    out[dy, j * C:(j + 1) * C, px * O:(px + 1) * O] = \
                        w[:, :, dy, dx].T
    return out


def _prep_weights(w1, w2, w3, w4, codebook, sigma, l1_w, l1_b, l2_w, l2_b):
    consts = {}
    # conv1: full (c,dy,j) contraction in one matmul: [54, 64=(px4,o16)]
    W1p = np.zeros((3, 3, 6, 4, 16), np.float32)  # c dy j px o
    for j in range(6):
        for px in range(4):
            dx = j - px
            if 0 <= dx <= 2:
                W1p[:, :, j, px, :] = w1[:, :, :, dx].transpose(1, 2, 0)
    consts["w1p"] = W1p.reshape(54, 64)

    consts["w2p"] = _conv_weight_mp(w2, P=4, J=6).transpose(1, 0, 2)  # [96,3,64]
    w3p = _conv_weight_mp(w3, P=2, J=4)                               # [3,64,48]
    w3pad = np.zeros((3, 64, 64), np.float32)
    w3pad[:, :, 0:48] = w3p
    consts["w3p"] = w3pad.transpose(1, 0, 2)                          # [64,3,64]
    w4p = _conv_weight_mp(w4, P=2, J=4)                               # [3,96,32]
    w4p0 = np.zeros((3, 96, 32), np.float32); w4p0[:, :, 0:16] = w4p[:, :, 0:16]
    w4p1 = np.zeros((3, 96, 32), np.float32); w4p1[:, :, 0:16] = w4p[:, :, 16:32]
    consts["w4p0"] = np.ascontiguousarray(w4p0.transpose(1, 0, 2))    # [96,3,32]
    consts["w4p1"] = np.ascontiguousarray(w4p1.transpose(1, 0, 2))

    c2 = (codebook * codebook).sum(axis=1)
    caug = np.zeros((32, KCB), np.float32)
    caug[0:16] = (2.0 * sigma[:, None] * codebook).T
    caug[16] = -sigma
    caug[17] = -sigma * c2
    consts["caug"] = np.tile(caug, (4, 1))                 # [128, 512]

    be = np.zeros((128, 4), np.float32)
    for s in range(4):
        be[32 * s:32 * s + 16, s] = 1.0
    consts["blocke"] = be

    consts["onesrows"] = np.ones((4, 512), np.float32)
    consts["ident4"] = np.eye(4, dtype=np.float32)

    l1s = l1_w / float(HP * HP)
    consts["m1w"] = np.ascontiguousarray(l1s.T)            # [512, 20]
    consts["m1b"] = l1_b.reshape(20, 1)
    consts["m2w"] = np.ascontiguousarray(l2_w.T)           # [20, 10]
    consts["m2b"] = l2_b.reshape(10, 1)
    return consts


def _prep_rhs1(x):
    """x [B,3,64,64] -> [54=(c,dy,j), B*64*16] fp32.
    rhs1[(c,dy,j),(b,y,xb)] = xpad[b, c, y+dy, 4*xb+j] (xpad has 1-px halo)."""
    B = x.shape[0]
    xpad = np.zeros((B, 3, H + 2, W + 2), np.float32)
    xpad[:, :, 1:-1, 1:-1] = x
    rhs1 = np.empty((3, 3, 6, B, H, 16), np.float32)
    xb_idx = 4 * np.arange(16)
    for dy in range(3):
        for j in range(6):
            rhs1[:, dy, j] = xpad[:, :, dy:dy + H, :][:, :, :, xb_idx + j] \
                .transpose(1, 0, 2, 3)
    return rhs1.reshape(54, B * H * 16)


# --------------------------------------------------------------------------
# device kernel
# --------------------------------------------------------------------------

def build_kernel(tc, outs, ins):
    nc = tc.nc
    with contextlib.ExitStack() as ctx:
        consts = ctx.enter_context(tc.tile_pool(name="consts", bufs=1))
        acts = ctx.enter_context(tc.tile_pool(name="acts", bufs=1))
        work = ctx.enter_context(tc.tile_pool(name="work", bufs=3))
        rbf = ctx.enter_context(tc.tile_pool(name="rbf", bufs=6))
        pc = ctx.enter_context(tc.tile_pool(name="pc", bufs=2, space="PSUM"))
        pa = ctx.enter_context(tc.tile_pool(name="pa", bufs=3, space="PSUM"))
        pf = ctx.enter_context(tc.tile_pool(name="pf", bufs=1, space="PSUM"))
        pp = ctx.enter_context(tc.tile_pool(name="pp", bufs=1, space="PSUM"))

        # ---- constants ----
        def cload(name, shape, dt=F16):
            t = consts.tile(shape, dt, tag=name, name=name)
            nc.sync.dma_start(t[:], ins[name][:])
            return t

        w1p = cload("w1p", [54, 64])
        w2p = cload("w2p", [96, 3, 64])
        w3p = cload("w3p", [64, 3, 64])
        w4p0 = cload("w4p0", [96, 3, 32])
        w4p1 = cload("w4p1", [96, 3, 32])
        caug = cload("caug", [128, KCB])
        blocke = cload("blocke", [128, 4])
        onesrows = cload("onesrows", [4, 512])
        ident4 = cload("ident4", [4, 4])
        m1b = cload("m1b", [20, 1], F32)
        m2w = cload("m2w", [20, 10])
        m2b = cload("m2b", [10, 1], F32)

        m1w = consts.tile([128, 4, 20], F16, tag="m1w", name="m1w")
        nc.sync.dma_start(m1w[:], ins["m1w"].rearrange("(q p) o -> p q o", p=128))

        rhs1 = consts.tile([54, B_CORE, H, 16], F16, tag="rhs1", name="rhs1")
        nc.sync.dma_start(rhs1[:], ins["rhs1"].rearrange(
            "k (b y xb) -> k b y xb", b=B_CORE, y=H, xb=16))

        # ---- persistent activation buffers ----
        act1 = [acts.tile([128, H, 16], F16, tag=f"act1_{p}", name=f"act1_{p}")
                for p in range(4)]
        rhs2 = [acts.tile([96, H + 2, 16], F16, tag=f"rhs2_{i}", name=f"rhs2_{i}")
                for i in range(4)]
        act2p = [acts.tile([128, HP + 2, 16], F16, tag=f"act2p_{p}",
                           name=f"act2p_{p}") for p in range(4)]
        rhs3 = [acts.tile([64, HP + 2, 16], F16, tag=f"rhs3_{i}", name=f"rhs3_{i}")
                for i in range(4)]
        act3 = [acts.tile([128, HP + 2, 16], F16, tag=f"act3_{p}",
                          name=f"act3_{p}") for p in range(4)]
        rhs4 = [acts.tile([96, HP + 2, 16], F16, tag=f"rhs4_{i}", name=f"rhs4_{i}")
                for i in range(4)]
        f_buf = [acts.tile([128, 512], F16, tag=f"f_{t}", name=f"f_{t}")
                 for t in range(4)]
        f2o = [acts.tile([8, 512], F16, tag=f"f2o_{t}", name=f"f2o_{t}")
               for t in range(4)]
        mlp_rhs = acts.tile([128, 4, B_CORE], F16, tag="mlp_rhs")

        for b in rhs2 + rhs3 + rhs4 + act2p + act3 + f_buf:
            nc.vector.memset(b[:], 0.0)
        for t in range(4):
            nc.sync.dma_start(f2o[t][4:8, :], onesrows[:])

        # ================= conv1 =================
        # act1 partition = 64*half + px*16 + o
        for pair in range(4):
            bA, bB = 2 * pair, 2 * pair + 1
            for h in range(2):
                ps = pc.tile([128, 32, 16], F32, tag="psc")
                nc.tensor.matmul(ps[0:64], w1p[:],
                                 rhs1[:, bA, 32 * h:32 * h + 32, :],
                                 start=True, stop=True)
                nc.tensor.matmul(ps[64:128], w1p[:],
                                 rhs1[:, bB, 32 * h:32 * h + 32, :],
                                 start=True, stop=True)
                nc.vector.tensor_scalar_max(
                    act1[pair][:, 32 * h:32 * h + 32, :], ps[:], 0.0)

        # ================= conv2 + pool =================
        # j -> (source px, xb shift): x = 4*xb_dst + j - 1 = 4*xb_src + px
        J2 = [(3, -1), (0, 0), (1, 0), (2, 0), (3, 0), (0, 1)]
        HSWAP = [(i + 16) % 32 for i in range(32)]  # swap px-pair halves
        for pair in range(4):
            for half in range(2):
                ioff = 64 * half
                r2 = rhs2[2 * (pair % 2) + half]
                eng2 = nc.gpsimd if half == 0 else nc.sync
                # j=1..4 read contiguous px-blocks 0..3 -> one DMA
                eng2.dma_start(r2[16:80, 1:65, :],
                               act1[pair][ioff:ioff + 64, :, :])
                for j, (pj, sh) in ((0, J2[0]), (5, J2[5])):
                    n = 16 - abs(sh)
                    d0, s0 = max(0, -sh), max(0, sh)
                    eng2.dma_start(
                        r2[16 * j:16 * j + 16, 1:65, d0:d0 + n],
                        act1[pair][ioff + 16 * pj:ioff + 16 * pj + 16, :,
                                   s0:s0 + n])
            for h in range(2):
                ps = pc.tile([128, 32, 16], F32, tag="psc")
                for half in range(2):
                    for dy in range(3):
                        nc.tensor.matmul(
                            ps[64 * half:64 * half + 64],
                            w2p[:, dy, :],
                            rhs2[2 * (pair % 2) + half][:, 32 * h + dy:32 * h + dy + 32, :],
                            start=(dy == 0), stop=(dy == 2),
                            tile_position=(0, 64 * half))
                # relu (psum fp32 -> sbuf fp16), then pool on SBUF
                t0 = work.tile([128, 32, 16], F16, tag="t0")
                nc.vector.tensor_scalar_max(t0[:], ps[:], 0.0)
                tp = work.tile([128, 16, 16], F16, tag="tp")
                v = t0[:].rearrange("p (Y yp) x -> p Y yp x", yp=2)
                nc.vector.tensor_tensor(tp[:], v[:, :, 0, :], v[:, :, 1, :],
                                        op=ALU.max)
                # pool-x: swap 16-blocks within 32-groups
                sh_t = work.tile([128, 16, 16], F16, tag="sh")
                nc.vector.stream_shuffle(sh_t[:], tp[:], HSWAP)
                nc.vector.tensor_tensor(
                    act2p[pair][:, 1 + 16 * h:17 + 16 * h, :],
                    sh_t[:], tp[:], op=ALU.max)

        # ================= conv3 =================
        # x3 = 2*xb3 + j - 1; source px-representative block in {0, 2}
        J3 = [(2, -1), (0, 0), (2, 0), (0, 1)]
        for pair in range(4):
            for half in range(2):
                ioff = 64 * half
                r3 = rhs3[2 * (pair % 2) + half]
                eng3 = nc.gpsimd if half == 0 else nc.sync
                for j, (pj, sh) in enumerate(J3):
                    n = 16 - abs(sh)
                    d0, s0 = max(0, -sh), max(0, sh)
                    eng3.dma_start(
                        r3[16 * j:16 * j + 16, :, d0:d0 + n],
                        act2p[pair][ioff + 16 * pj:ioff + 16 * pj + 16, :,
                                    s0:s0 + n])
            ps = pc.tile([128, 32, 16], F32, tag="psc")
            for half in range(2):
                for dy in range(3):
                    nc.tensor.matmul(
                        ps[64 * half:64 * half + 64],
                        w3p[:, dy, :],
                        rhs3[2 * (pair % 2) + half][:, dy:dy + 32, :],
                        start=(dy == 0), stop=(dy == 2),
                        tile_position=(0, 64 * half))
            nc.vector.tensor_scalar_max(act3[pair][:, 1:33, :], ps[:], 0.0)

        # ================= conv4 + f' assembly =================
        # act3 partition = 64*half + px*24 + o (px in {0,1})
        J4 = [(1, -1), (0, 0), (1, 0), (0, 1)]
        for t in range(4):
            fb = f_buf[t]
            ps4 = pa.tile([128, 512], F32, tag="psa")
            for i in range(2):
                img = 2 * t + i
                pair, half = img // 2, img % 2
                ioff = 64 * half
                r4 = rhs4[2 * (t % 2) + i]
                eng4 = nc.gpsimd if i == 0 else nc.sync
                # j=1,2 read contiguous px-blocks 0,1 -> one DMA
                eng4.dma_start(r4[24:72, :, :],
                               act3[pair][ioff:ioff + 48, :, :])
                for j, (pj, sh) in ((0, J4[0]), (3, J4[3])):
                    n = 16 - abs(sh)
                    d0, s0 = max(0, -sh), max(0, sh)
                    eng4.dma_start(
                        r4[24 * j:24 * j + 24, :, d0:d0 + n],
                        act3[pair][ioff + 24 * pj:ioff + 24 * pj + 24, :,
                                   s0:s0 + n])
                for px in range(2):
                    s = 2 * i + px
                    w4 = w4p0 if px == 0 else w4p1
                    for dy in range(3):
                        nc.tensor.matmul(
                            ps4[32 * s:32 * s + 32, :],
                            w4[:, dy, :],
                            r4[:, dy:dy + 32, :].rearrange("p y x -> p (y x)"),
                            start=(dy == 0), stop=(dy == 2),
                            tile_position=(0, 32 * s))
            nc.vector.tensor_scalar_max(fb[:], ps4[:], 0.0)
            fsq = work.tile([128, 512], F16, tag="fsq")
            nc.vector.tensor_mul(fsq[:], fb[:], fb[:])
            psf = pf.tile([4, 512], F32, tag="psmall")
            nc.tensor.matmul(psf[:], blocke[:], fsq[:], start=True, stop=True)
            nc.vector.tensor_copy(f2o[t][0:4, :], psf[:])
            for s_ in range(4):
                nc.sync.dma_start(fb[32 * s_ + 16:32 * s_ + 17, :],
                                  f2o[t][s_:s_ + 1, :])
                nc.sync.dma_start(fb[32 * s_ + 17:32 * s_ + 18, :],
                                  f2o[t][4 + s_:5 + s_, :])

        # ================= RBF =================
        ppool = [pp.tile([128, 512], F32, tag=f"ppool{g}", name=f"ppool{g}")
                 for g in range(2)]
        for t in range(4):
            fb = f_buf[t]
            for i in range(2):
                img = 2 * t + i
                pg, prow = img // 4, 32 * (img % 4)
                for px in range(2):
                    s = 2 * i + px
                    S_slot = rbf.tile([128, 4], F32, tag="S")
                    a_tiles = []
                    for q in range(4):
                        psa = pa.tile([128, 512], F32, tag="psa")
                        nc.tensor.matmul(
                            psa[:],
                            fb[32 * s:32 * s + 18, 128 * q:128 * q + 128],
                            caug[32 * s:32 * s + 18, :],
                            start=True, stop=True,
                            tile_position=(32 * s, 0))
                        at = rbf.tile([128, 512], F16, tag="a")
                        nc.scalar.activation(at[:], psa[:], AF.Exp,
                                             accum_out=S_slot[:, q:q + 1])
                        a_tiles.append(at)
                    R_pad = rbf.tile([128, 128], F16, tag="R")
                    nc.vector.memset(R_pad[:], 0.0)
                    with nc.allow_low_precision(reason="R feeds fp16 matmul"):
                        nc.vector.reciprocal(R_pad[:, 0:128:32], S_slot[:])
                    for q in range(4):
                        nc.tensor.matmul(
                            ppool[pg][prow:prow + 32, :],
                            R_pad[:, 32 * q:32 * q + 32], a_tiles[q][:],
                            start=(px == 0 and q == 0),
                            stop=(px == 1 and q == 3),
                            tile_position=(0, prow))
        for pg in range(2):
            ptmp = work.tile([128, 512], F16, tag="ptmp")
            with nc.allow_low_precision(reason="pooled to fp16 for MLP"):
                nc.vector.tensor_copy(ptmp[:], ppool[pg][:])
            pslim = work.tile([4, 512], F16, tag="pslim")
            for i in range(4):
                nc.sync.dma_start(pslim[i:i + 1, :],
                                  ptmp[32 * i:32 * i + 1, :])
            for q in range(4):
                ptr = pf.tile([128, 4], F16, tag="psmall")
                nc.tensor.transpose(ptr[:], pslim[:, 128 * q:128 * q + 128],
                                    ident4[:])
                nc.vector.tensor_copy(mlp_rhs[:, q, 4 * pg:4 * pg + 4], ptr[:])

        # ================= MLP =================
        psz = pf.tile([20, B_CORE], F32, tag="psmall")
        for q in range(4):
            nc.tensor.matmul(psz[:], m1w[:, q, :], mlp_rhs[:, q, :],
                             start=(q == 0), stop=(q == 3))
        z = work.tile([20, B_CORE], F16, tag="z")
        nc.scalar.activation(z[:], psz[:], AF.Relu, bias=m1b[:])
        pso = pf.tile([10, B_CORE], F32, tag="psmall")
        nc.tensor.matmul(pso[:], m2w[:], z[:], start=True, stop=True)
        ot = work.tile([10, B_CORE], F32, tag="ot")
        nc.scalar.activation(ot[:], pso[:], AF.Identity, bias=m2b[:])
        nc.sync.dma_start(outs["out"].rearrange("b o -> o b"), ot[:])


# --------------------------------------------------------------------------
# entry point
# --------------------------------------------------------------------------

_CACHE = {}

IN_SPECS = {
    "rhs1": ([54, B_CORE * H * 16], F16),
    "w1p": ([54, 64], F16),
    "w2p": ([96, 3, 64], F16),
    "w3p": ([64, 3, 64], F16),
    "w4p0": ([96, 3, 32], F16),
    "w4p1": ([96, 3, 32], F16),
    "caug": ([128, KCB], F16),
    "blocke": ([128, 4], F16),
    "onesrows": ([4, 512], F16),
    "ident4": ([4, 4], F16),
    "m1w": ([KCB, 20], F16),
    "m1b": ([20, 1], F32),
    "m2w": ([20, 10], F16),
    "m2b": ([10, 1], F32),
}


def get_compiled():
    if "nc" not in _CACHE:
        nc = bacc.Bacc("TRN2", target_bir_lowering=False, debug=False,
                       num_devices=N_CORES)
        ins = {k: nc.dram_tensor(k, shp, dt, kind="ExternalInput").ap()
               for k, (shp, dt) in IN_SPECS.items()}
        outs = {"out": nc.dram_tensor("out", [B_CORE, 10], F32,
                                      kind="ExternalOutput").ap()}
        with tile.TileContext(nc) as tc:
            build_kernel(tc, outs, ins)
        nc.compile()
        _CACHE.update(nc=nc, ins=ins, outs=outs)
    return _CACHE["nc"]


def make_in_maps(x, w1, b1, w2, b2, w3, b3, w4, b4, codebook, sigma,
                 l1_w, l1_b, l2_w, l2_b):
    for b in (b1, b2, b3, b4):
        assert np.abs(np.asarray(b)).max() == 0.0, "conv biases assumed zero"
    consts = _prep_weights(np.asarray(w1, np.float32), np.asarray(w2, np.float32),
                           np.asarray(w3, np.float32), np.asarray(w4, np.float32),
                           np.asarray(codebook, np.float32),
                           np.asarray(sigma, np.float32),
                           np.asarray(l1_w, np.float32),
                           np.asarray(l1_b, np.float32),
                           np.asarray(l2_w, np.float32),
                           np.asarray(l2_b, np.float32))
    cm = {}
    for k, v in consts.items():
        dt = IN_SPECS[k][1]
        cm[k] = v.astype(np.float16 if dt == F16 else np.float32)
    x = np.asarray(x, np.float32)
    in_maps = []
    for c in range(N_CORES):
        rhs1 = _prep_rhs1(x[B_CORE * c:B_CORE * (c + 1)]).astype(np.float16)
        m = dict(cm)
        m["rhs1"] = rhs1
        in_maps.append(m)
    return in_maps


def kernel(x, w1, b1, w2, b2, w3, b3, w4, b4, codebook, sigma,
           l1_w, l1_b, l2_w, l2_b):
    nc = get_compiled()
    in_maps = make_in_maps(x, w1, b1, w2, b2, w3, b3, w4, b4, codebook,
                           sigma, l1_w, l1_b, l2_w, l2_b)
    res = bass_utils.run_bass_kernel_spmd(nc, in_maps, list(range(N_CORES)))
    out = np.concatenate([res.results[c]["out"] for c in range(N_CORES)],
                         axis=0)
    return out.astype(np.float32)



# revision 6
# speedup vs baseline: 1.3925x; 1.3925x over previous
"""Trainium2 Bass kernel for nn_Boftrainer_48284022342309 (vq_codebook).

Data-parallel over batch: 64 images -> 8 cores x 8 images each.

Redesign vs the v1 kernel: each conv layer's matmul writes the NEXT
layer's im2col layout directly by extending the stationary operand with
shifted-tap output copies (extra output partitions).  This removes the
SB->SB rearrangement DMAs for conv2 and conv4 entirely; conv3's rhs
needs only 3 contiguous-block DMAs per image (middle + two flat-shifted
copies).  All DMAs are large and spread across the sync/scalar/gpsimd
queues.  The conv trunk of image-pair t+1 is interleaved under the RBF
(exp-bound) phase of pair t, so the scalar engine's ~51us of exp work
bounds the kernel.

Layouts (per core, 8 images):
  rhs1  [72=(c3,dy3,j8), img8, y64, xb16]  host im2col, fp16
        elem = xpad[c, y+dy-1, 4xb+j-2]
  conv1 out = rhs2 [96=(j6,c16), img8, y66(halo), xb16]; j-block j
        carries conv1 output at x = 4xb+j-1 (relu'd).
  conv2 out [128=(img2 x px4 x o16), y32, xb16] -> relu -> pool ->
        act2p [128, pair4, y34(halo), xb16] (pooled, px pairs dup'd)
  rhs3  [96=(jt6,c16), img8, y34, xb16]; block jt carries pooled x~ =
        2xb+jt-2; built from act2p[64i+16:64i+48] by 3 DMAs (shift -1/0/+1)
  conv3 out = rhs4 [96=(j4,c24)+..., img8, y34, xb16] extended px' in
        {-1..2} = conv4's im2col directly.
  conv4 out ps4 [128=(slot4 x 32), 512]; slot s=2i+px, rows 32s..32s+16
        = f, rows +16..+32 zero; relu -> fb; f2 via blockeE matmul +
        strided-partition copy; ones row via gpsimd memset.
  RBF   per slot: 4 dist matmuls vs caug -> ACT Exp (accum S) -> recip
        -> 4 pool matmuls accumulating pooled in PSUM per image band.
  MLP   pooled -> transposes -> two small matmuls with ACT biases.
"""
import sys
sys.path.insert(0, "/opt/trn_rl_repo")

import contextlib

import numpy as np

import concourse.bass as bass
import concourse.tile as tile
from concourse import bacc, mybir
from concourse import bass_utils

F32 = mybir.dt.float32
F16 = mybir.dt.float16

N_CORES = 8
B_CORE = 8
H = W = 64
KCB = 512

AF = mybir.ActivationFunctionType
ALU = mybir.AluOpType

# cpack column offsets
OW1, OW2, OW3, OW40, OW41 = 0, 96, 288, 672, 768
OCAUG, OBLK, OIF, OSEL, OONE = 864, 1376, 1504, 1632, 1760
OM1W, OM2W, OID = 2272, 2352, 2362
CPW = 2366
# conv1 output j-block slot order: slot k holds j-block J1PERM[k]
J1PERM = [0, 1, 5, 2, 3, 4]
J1SLOT = [J1PERM.index(j) for j in range(6)]


# --------------------------------------------------------------------------
# host-side transforms (numpy)
# --------------------------------------------------------------------------

def _conv_weight_mp(w, P, J):
    """w [O,C,3,3] -> per-dy lhsT [J*C, P*O]."""
    O, C = w.shape[0], w.shape[1]
    out = np.zeros((3, J * C, P * O), np.float32)
    for dy in range(3):
        for j in range(J):
            for px in range(P):
                dx = j - px
                if 0 <= dx <= 2:
                    out[dy, j * C:(j + 1) * C, px * O:(px + 1) * O] = \
                        w[:, :, dy, dx].T
    return out


def _prep_weights(w1, w2, w3, w4, codebook, sigma, l1_w, l1_b, l2_w, l2_b):
    cp = np.zeros((128, CPW), np.float32)

    # conv1 extended: [72=(c,dy,j8), 96=(slot6, o16)], slot order J1PERM
    # (keeps both x-edge blocks at legal memset partition starts 0 and 32)
    W1E = np.zeros((3, 3, 8, 6, 16), np.float32)
    for jidx in range(8):
        for jout in range(6):
            dx = jidx - jout
            if 0 <= dx <= 2:
                W1E[:, :, jidx, J1SLOT[jout], :] = \
                    w1[:, :, :, dx].transpose(1, 2, 0)
    cp[0:72, OW1:OW1 + 96] = W1E.reshape(72, 96)

    w2p = _conv_weight_mp(w2, P=4, J=6)                       # [3, 96, 64]
    w2p = w2p.reshape(3, 6, 16, 64)[:, J1PERM].reshape(3, 96, 64)
    cp[0:96, OW2:OW2 + 192] = w2p.transpose(1, 0, 2).reshape(96, 192)

    # conv3 extended: [(jt6,c16), 3, (b4=px'+1, o24 pad32)]
    W3E = np.zeros((3, 6, 16, 4, 32), np.float32)
    for jt in range(6):
        for b_ in range(4):
            dx = jt - b_
            if 0 <= dx <= 2:
                W3E[:, jt, :, b_, 0:24] = w3[:, :, :, dx].transpose(2, 1, 0)
    cp[0:96, OW3:OW3 + 384] = \
        W3E.reshape(3, 96, 128).transpose(1, 0, 2).reshape(96, 384)

    w4p = _conv_weight_mp(w4, P=2, J=4)                       # [3, 96, 32]
    w4p = w4p.reshape(3, 4, 24, 32)
    w4pp = np.zeros((3, 4, 32, 32), np.float32)
    w4pp[:, :, 0:24, :] = w4p
    w4pp = w4pp.reshape(3, 128, 32)                 # rows (j4, c24+8pad)
    w4p0 = np.zeros((3, 128, 32), np.float32); w4p0[:, :, 0:16] = w4pp[:, :, 0:16]
    w4p1 = np.zeros((3, 128, 32), np.float32); w4p1[:, :, 0:16] = w4pp[:, :, 16:32]
    cp[:, OW40:OW40 + 96] = w4p0.transpose(1, 0, 2).reshape(128, 96)
    cp[:, OW41:OW41 + 96] = w4p1.transpose(1, 0, 2).reshape(128, 96)

    c2 = (codebook * codebook).sum(axis=1)
    caug = np.zeros((32, KCB), np.float32)
    caug[0:16] = (2.0 * sigma[:, None] * codebook).T
    caug[16] = -sigma
    caug[17] = -sigma * c2
    cp[:, OCAUG:OCAUG + KCB] = np.tile(caug, (4, 1))

    blkE = np.zeros((128, 128), np.float32)
    for s in range(4):
        blkE[32 * s:32 * s + 16, 32 * s + 16] = 1.0
    cp[:, OBLK:OBLK + 128] = blkE
    If = np.zeros((128, 128), np.float32)
    for s in range(4):
        for c_ in range(16):
            If[32 * s + c_, 32 * s + c_] = 1.0
    cp[:, OIF:OIF + 128] = If
    cp[0, OSEL + 17:OSEL + 128:32] = 1.0
    cp[0, OONE:OONE + 512] = 1.0

    l1s = l1_w / 1024.0
    cp[:, OM1W:OM1W + 80] = \
        np.ascontiguousarray(l1s.T).reshape(4, 128, 20).transpose(1, 0, 2) \
        .reshape(128, 80)
    cp[0:20, OM2W:OM2W + 10] = l2_w.T
    cp[0:4, OID:OID + 4] = np.eye(4, dtype=np.float32)

    biasp = np.zeros((20, 2), np.float32)
    biasp[0:20, 0] = l1_b
    biasp[0:10, 1] = l2_b
    return cp.astype(np.float16), biasp


def _prep_rhs1(x):
    """x [B,3,64,64] -> [72=(c,dy,j8), B*64*16] fp16."""
    B = x.shape[0]
    xpad = np.zeros((B, 3, H + 2, W + 4), np.float32)
    xpad[:, :, 1:-1, 2:-2] = x
    rhs1 = np.empty((3, 3, 8, B, H, 16), np.float32)
    xb4 = 4 * np.arange(16)
    for dy in range(3):
        for j in range(8):
            rhs1[:, dy, j] = xpad[:, :, dy:dy + H, :][:, :, :, xb4 + j] \
                .transpose(1, 0, 2, 3)
    return rhs1.reshape(72, B * H * 16).astype(np.float16)


# --------------------------------------------------------------------------
# device kernel
# --------------------------------------------------------------------------

HSWAP = [(i + 16) % 32 for i in range(32)]


def build_kernel(tc, outs, ins):
    nc = tc.nc
    with contextlib.ExitStack() as ctx:
        consts = ctx.enter_context(tc.tile_pool(name="consts", bufs=1))
        acts = ctx.enter_context(tc.tile_pool(name="acts", bufs=1))
        work = ctx.enter_context(tc.tile_pool(name="work", bufs=3))
        apool = ctx.enter_context(tc.tile_pool(name="apool", bufs=8))
        spool = ctx.enter_context(tc.tile_pool(name="spool", bufs=4))
        fepool = ctx.enter_context(tc.tile_pool(name="fepool", bufs=2))
        pc = ctx.enter_context(tc.tile_pool(name="pc", bufs=2, space="PSUM"))
        pm = ctx.enter_context(tc.tile_pool(name="pm", bufs=4, space="PSUM"))
        pp = ctx.enter_context(tc.tile_pool(name="pp", bufs=1, space="PSUM"))
        pf = ctx.enter_context(tc.tile_pool(name="pf", bufs=1, space="PSUM"))

        # ---- constants + input ----
        cp = consts.tile([128, CPW], F16, tag="cp", name="cp")
        nc.scalar.dma_start(cp[:], ins["cpack"][:])
        biasp = consts.tile([20, 2], F32, tag="biasp", name="biasp")
        nc.gpsimd.dma_start(biasp[:], ins["biasp"][:])
        rhs1 = consts.tile([72, B_CORE, H, 16], F16, tag="rhs1", name="rhs1")
        rhs1_d = ins["rhs1"].rearrange("k (b y x) -> k b y x", b=B_CORE, y=H,
                                       x=16)
        nc.sync.dma_start(rhs1[:, 0:4], rhs1_d[:, 0:4])
        nc.sync.dma_start(rhs1[:, 4:8], rhs1_d[:, 4:8])

        w1v = cp[0:72, OW1:OW1 + 96]
        w2v = cp[0:96, OW2:OW2 + 192].rearrange("p (d m) -> p d m", d=3)
        w3v = cp[0:96, OW3:OW3 + 384].rearrange("p (d m) -> p d m", d=3)
        w40v = cp[:, OW40:OW40 + 96].rearrange("p (d m) -> p d m", d=3)
        w41v = cp[:, OW41:OW41 + 96].rearrange("p (d m) -> p d m", d=3)
        caugv = cp[:, OCAUG:OCAUG + KCB]
        blkv = cp[:, OBLK:OBLK + 128]
        ifv = cp[:, OIF:OIF + 128]
        oselv = cp[0:1, OSEL:OSEL + 128]
        onerv = cp[0:1, OONE:OONE + 512]
        m1wv = cp[:, OM1W:OM1W + 80].rearrange("p (q o) -> p q o", q=4)
        m2wv = cp[0:20, OM2W:OM2W + 10]
        id4v = cp[0:4, OID:OID + 4]

        # ---- persistent activation tiles ----
        rhs2 = acts.tile([96, B_CORE, 66, 16], F16, tag="rhs2", name="rhs2")
        act2p = acts.tile([128, 4, 34, 16], F16, tag="act2p", name="act2p")
        rhs3 = acts.tile([96, B_CORE, 34, 16], F16, tag="rhs3", name="rhs3")
        rhs4 = acts.tile([128, B_CORE, 34, 16], F16, tag="rhs4", name="rhs4")
        fb = [acts.tile([128, KCB], F16, tag=f"fb{t}", name=f"fb{t}")
              for t in range(4)]
        Rp = [acts.tile([128, 128], F16, tag=f"Rp{i}", name=f"Rp{i}")
              for i in range(4)]
        ptmp = acts.tile([128, 2, KCB], F16, tag="ptmp", name="ptmp")
        pslim = acts.tile([4, 2, KCB], F16, tag="pslim", name="pslim")
        mlp_rhs = acts.tile([128, 4, B_CORE], F16, tag="mlp_rhs",
                            name="mlp_rhs")

        r3f = rhs3.rearrange("p b y x -> p b (y x)")
        a2f = act2p.rearrange("p b y x -> p b (y x)")

        # ---- one-time halo zeroing ----
        nc.vector.memset(rhs2[:, :, 0:66:65, :], 0.0)
        nc.vector.memset(act2p[:, :, 0:34:33, :], 0.0)
        nc.vector.memset(rhs4[:, :, 0:34:33, :], 0.0)
        for i in range(4):
            nc.vector.memset(Rp[i][:], 0.0)

        # ---- trunk stages ----
        def t_conv1(t):
            for i in range(2):
                img = 2 * t + i
                for h in range(2):
                    ps = pc.tile([128, 512], F32, tag="pc")
                    psv = ps.rearrange("p (y x) -> p y x", x=16)
                    nc.tensor.matmul(psv[0:96], w1v[:],
                                     rhs1[:, img, 32 * h:32 * h + 32, :],
                                     start=True, stop=True,
                                     tile_position=(0, 0))
                    nc.vector.tensor_scalar_max(
                        rhs2[:, img, 1 + 32 * h:33 + 32 * h, :], psv[0:96],
                        0.0)
                # invalid extended outputs: jout0 @ x=-1, jout5 @ x=64
                nc.vector.memset(rhs2[0:16, img, 1:65, 0:1], 0.0)
                nc.vector.memset(rhs2[32:48, img, 1:65, 15:16], 0.0)

        def t_conv2(t):
            for h in range(2):
                ps = pc.tile([128, 512], F32, tag="pc")
                psv = ps.rearrange("p (y x) -> p y x", x=16)
                for i in range(2):
                    img = 2 * t + i
                    for dy in range(3):
                        nc.tensor.matmul(
                            psv[64 * i:64 * i + 64], w2v[:, dy, :],
                            rhs2[:, img, 32 * h + dy:32 * h + dy + 32, :],
                            start=(dy == 0), stop=(dy == 2),
                            tile_position=(0, 64 * i))
                t0 = work.tile([128, 32, 16], F16, tag="t0")
                nc.vector.tensor_scalar_max(t0[:], psv[:], 0.0)
                tp = work.tile([128, 16, 16], F16, tag="tp")
                v = t0[:].rearrange("p (Y yp) x -> p Y yp x", yp=2)
                nc.vector.tensor_tensor(tp[:], v[:, :, 0, :], v[:, :, 1, :],
                                        op=ALU.max)
                sh = work.tile([128, 16, 16], F16, tag="sh")
                nc.vector.stream_shuffle(sh[:], tp[:], HSWAP)
                nc.vector.tensor_tensor(
                    act2p[:, t, 1 + 16 * h:17 + 16 * h, :], sh[:], tp[:],
                    op=ALU.max)

        def t_rhs3(t):
            for i in range(2):
                img = 2 * t + i
                s0 = 64 * i + 16
                eng = nc.sync if i == 0 else nc.gpsimd
                eng.dma_start(rhs3[32:64, img, :, :],
                              act2p[s0:s0 + 32, t, :, :])
                eng.dma_start(r3f[0:32, img, 1:544], a2f[s0:s0 + 32, t, 0:543])
                eng.dma_start(r3f[64:96, img, 0:543], a2f[s0:s0 + 32, t, 1:544])
                nc.gpsimd.memset(rhs3[0:32, img, :, 0:1], 0.0)
                nc.gpsimd.memset(rhs3[64:96, img, :, 15:16], 0.0)

        def t_conv3(t):
            for i in range(2):
                img = 2 * t + i
                ps = pc.tile([128, 512], F32, tag="pc")
                psv = ps.rearrange("p (y x) -> p y x", x=16)
                for dy in range(3):
                    nc.tensor.matmul(psv[:], w3v[:, dy, :],
                                     rhs3[:, img, dy:dy + 32, :],
                                     start=(dy == 0), stop=(dy == 2),
                                     tile_position=(0, 0))
                nc.vector.tensor_scalar_max(rhs4[:, img, 1:33, :], psv[:],
                                            0.0)
                # conv4 padding: x~=-1 (block j0 @ xb0), x~=32 (j3 @ xb15)
                nc.vector.memset(rhs4[0:32, img, :, 0:1], 0.0)
                nc.vector.memset(rhs4[96:128, img, :, 15:16], 0.0)

        def t_conv4(t):
            ps4 = pm.tile([128, KCB], F32, tag="pm")
            for i in range(2):
                img = 2 * t + i
                r4 = rhs4[:, img, :, :]
                for px in range(2):
                    s = 2 * i + px
                    w4 = w40v if px == 0 else w41v
                    for dy in range(3):
                        nc.tensor.matmul(
                            ps4[32 * s:32 * s + 32, :], w4[:, dy, :],
                            r4[:, dy:dy + 32, :].rearrange("p y x -> p (y x)"),
                            start=(dy == 0), stop=(dy == 2),
                            tile_position=(0, 32 * s))
            nc.vector.tensor_scalar_max(fb[t][:], ps4[:], 0.0)
            fsq = work.tile([128, KCB], F16, tag="fsq")
            nc.vector.tensor_mul(fsq[:], fb[t][:], fb[t][:])
            # psE = f' assembled in PSUM: f rows (I_f @ fb), f2 rows
            # (blockeE @ fsq), ones rows (onesel @ onesrow)
            psE = pm.tile([128, KCB], F32, tag="pm")
            nc.tensor.matmul(psE[:], blkv[:], fsq[:], start=True, stop=False,
                             tile_position=(0, 0))
            nc.tensor.matmul(psE[:], ifv[:], fb[t][:], start=False,
                             stop=False, tile_position=(0, 0))
            nc.tensor.matmul(psE[:], oselv[:], onerv[:], start=False,
                             stop=True, tile_position=(0, 0))
            fE = fepool.tile([128, KCB], F16, tag="fE")
            with nc.allow_low_precision(reason="f' feeds fp16 matmul"):
                nc.vector.tensor_copy(fE[:], psE[:])
            return fE

        def trunk(t):
            t_conv1(t); t_conv2(t); t_rhs3(t); t_conv3(t)
            return t_conv4(t)

        # ---- RBF slot ----
        def rbf_slot(fE, s, img, ppool_g):
            px = s % 2
            prow = 32 * (img % 4)
            S = spool.tile([128, 4], F32, tag="S")
            ats = []
            for q in range(4):
                psa = pm.tile([128, KCB], F32, tag="pm")
                nc.tensor.matmul(
                    psa[:], fE[32 * s:32 * s + 18, 128 * q:128 * q + 128],
                    caugv[32 * s:32 * s + 18, :], start=True, stop=True,
                    tile_position=(32 * s, 0))
                at = apool.tile([128, KCB], F16, tag="a")
                nc.scalar.activation(at[:], psa[:], AF.Exp,
                                     accum_out=S[:, q:q + 1])
                ats.append(at)
            ri = Rp[s]
            with nc.allow_low_precision(reason="R feeds fp16 matmul"):
                nc.vector.reciprocal(ri[:, 0:128:32], S[:])
            for q in range(4):
                nc.tensor.matmul(
                    ppool_g[prow:prow + 32, :], ri[:, 32 * q:32 * q + 32],
                    ats[q][:], start=(px == 0 and q == 0),
                    stop=(px == 1 and q == 3), tile_position=(0, prow))

        def finalize(g, ppool_g):
            with nc.allow_low_precision(reason="pooled to fp16 for MLP"):
                nc.vector.tensor_copy(ptmp[:, g, :], ppool_g[:])
            nc.sync.dma_start(pslim[0:4, g, :], ptmp[0:128:32, g, :])
            for q in range(4):
                ptr = pf.tile([128, 4], F16, tag="pf")
                nc.tensor.transpose(ptr[:],
                                    pslim[0:4, g, 128 * q:128 * q + 128],
                                    id4v[:])
                nc.vector.tensor_copy(mlp_rhs[:, q, 4 * g:4 * g + 4], ptr[:])

        # ---- emission schedule ----
        fE_cur = trunk(0)
        fE_next = None
        ppool_g = None
        for t in range(4):
            if t % 2 == 0:
                ppool_g = pp.tile([128, KCB], F32, tag="ppool")
            for s in range(4):
                rbf_slot(fE_cur, s, 2 * t + s // 2, ppool_g)
                if t < 3:
                    if s == 0:
                        t_conv1(t + 1)
                    elif s == 1:
                        t_conv2(t + 1)
                    elif s == 2:
                        t_rhs3(t + 1); t_conv3(t + 1)
                    else:
                        fE_next = t_conv4(t + 1)
            fE_cur = fE_next
            if t == 1:
                finalize(0, ppool_g)
            if t == 3:
                finalize(1, ppool_g)

        # ---- MLP ----
        psz = pf.tile([20, B_CORE], F32, tag="pf")
        for q in range(4):
            nc.tensor.matmul(psz[:], m1wv[:, q, :], mlp_rhs[:, q, :],
                             start=(q == 0), stop=(q == 3))
        z = work.tile([20, B_CORE], F16, tag="z")
        nc.scalar.activation(z[:], psz[:], AF.Relu, bias=biasp[0:20, 0:1])
        pso = pf.tile([10, B_CORE], F32, tag="pf")
        nc.tensor.matmul(pso[:], m2wv[:], z[:], start=True, stop=True)
        ot = work.tile([10, B_CORE], F32, tag="ot")
        nc.scalar.activation(ot[:], pso[:], AF.Identity, bias=biasp[0:10, 1:2])
        nc.sync.dma_start(outs["out"].rearrange("b o -> o b"), ot[:])


# --------------------------------------------------------------------------
# entry point
# --------------------------------------------------------------------------

_CACHE = {}

IN_SPECS = {
    "cpack": ([128, CPW], F16),
    "biasp": ([20, 2], F32),
    "rhs1": ([72, B_CORE * H * 16], F16),
}


def get_compiled():
    if "nc" not in _CACHE:
        nc = bacc.Bacc("TRN2", target_bir_lowering=False, debug=False,
                       num_devices=N_CORES)
        ins = {k: nc.dram_tensor(k, shp, dt, kind="ExternalInput").ap()
               for k, (shp, dt) in IN_SPECS.items()}
        outs = {"out": nc.dram_tensor("out", [B_CORE, 10], F32,
                                      kind="ExternalOutput").ap()}
        with tile.TileContext(nc) as tc:
            build_kernel(tc, outs, ins)
        nc.compile()
        _CACHE.update(nc=nc, ins=ins, outs=outs)
    return _CACHE["nc"]


def make_in_maps(x, w1, b1, w2, b2, w3, b3, w4, b4, codebook, sigma,
                 l1_w, l1_b, l2_w, l2_b):
    for b in (b1, b2, b3, b4):
        assert np.abs(np.asarray(b)).max() == 0.0, "conv biases assumed zero"
    cp, biasp = _prep_weights(
        np.asarray(w1, np.float32), np.asarray(w2, np.float32),
        np.asarray(w3, np.float32), np.asarray(w4, np.float32),
        np.asarray(codebook, np.float32), np.asarray(sigma, np.float32),
        np.asarray(l1_w, np.float32), np.asarray(l1_b, np.float32),
        np.asarray(l2_w, np.float32), np.asarray(l2_b, np.float32))
    x = np.asarray(x, np.float32)
    in_maps = []
    for c in range(N_CORES):
        in_maps.append({
            "cpack": cp,
            "biasp": biasp,
            "rhs1": _prep_rhs1(x[B_CORE * c:B_CORE * (c + 1)]),
        })
    return in_maps


def kernel(x, w1, b1, w2, b2, w3, b3, w4, b4, codebook, sigma,
           l1_w, l1_b, l2_w, l2_b):
    nc = get_compiled()
    in_maps = make_in_maps(x, w1, b1, w2, b2, w3, b3, w4, b4, codebook,
                           sigma, l1_w, l1_b, l2_w, l2_b)
    res = bass_utils.run_bass_kernel_spmd(nc, in_maps, list(range(N_CORES)))
    out = np.concatenate([res.results[c]["out"] for c in range(N_CORES)],
                         axis=0)
    return out.astype(np.float32)
